# revision 12
# baseline (speedup 1.0000x reference)
"""Trainium2 Bass kernel for the Camera (3D Gaussian screen projection) problem.

Computes, for N=2,000,000 gaussians:
  - pos2d [N,3]  screen-space positions (culled -> 0)
  - cov2d [N,2,2] projected 2x2 covariances (culled -> 0)
  - mask  [N]    visibility mask

Sharded data-parallel over the gaussian axis across 8 NeuronCores.
"""

import os
import sys

import numpy as np


def _ensure_import_path():
    for p in ("/opt/trn_rl_repo", os.path.expanduser("~/.axon_site/_ro/trn_rl_repo")):
        if os.path.isdir(p) and p not in sys.path:
            sys.path.insert(0, p)


try:
    import concourse.bass as bass  # noqa: F401
except ImportError:
    _ensure_import_path()
    import concourse.bass as bass

import concourse.tile as tile
from concourse import mybir
from concourse.bass_utils import run_bass_kernel_spmd

N_CORES = 8
P = 128          # SBUF partitions
F = 489          # free-dim columns per tile (elements per partition per tile)
NT = 4           # tiles per core
NC_ELEMS = P * F * NT          # 250,368 elements per core (padded)
N_PAD = N_CORES * NC_ELEMS     # 2,002,944

WIDTH, HEIGHT = 1920.0, 1080.0
RELAX = 1.3

_dt = mybir.dt.float32
_u8 = mybir.dt.uint8
_Alu = mybir.AluOpType
_Act = mybir.ActivationFunctionType


_CTRL_INSTS = ("InstDrain", "InstNoOp", "InstEventSemaphore", "InstHalt")


def _split_excess_waits(nc):
    """Walrus in this toolchain limits sem-wait commands per instruction
    (1 for CTRL-type instructions like Drain, 2 for compute). Move excess
    waits onto InstEventSemaphore instructions inserted immediately before
    the offending instruction (same engine, so ordering is preserved)."""
    counter = [0]
    for bb in nc.main_func.blocks:
        il = list(bb.instructions)
        out = []
        changed = False
        for ins in il:
            si = ins.sync_info
            max_waits = 1
            if si is not None and si.on_wait and len(si.on_wait) > max_waits:
                waits = list(si.on_wait)
                excess, keep = waits[:-max_waits], waits[-max_waits:]
                for wchunk in excess:
                    ev = mybir.InstEventSemaphore(
                        name=f"wait_split_{counter[0]}",
                        engine=ins.engine,
                        sync_info=mybir.SyncInfo(on_wait=[wchunk], on_update=[]),
                    )
                    counter[0] += 1
                    out.append(ev)
                ins.sync_info = mybir.SyncInfo(
                    on_wait=keep, on_update=list(si.on_update)
                )
                changed = True
            out.append(ins)
        if changed:
            bb.instructions = out
    return counter[0]


def _camera_consts(world2model, projection):
    """Validate the expected camera structure and extract scalar constants."""
    W = np.asarray(world2model, dtype=np.float32)
    Pm = np.asarray(projection, dtype=np.float32)
    assert W.shape == (4, 4) and Pm.shape == (4, 4)
    # rotation must be identity, translation only along z
    assert np.array_equal(W[:3, :3], np.eye(3, dtype=np.float32)), W
    assert W[0, 3] == 0.0 and W[1, 3] == 0.0, W
    assert np.array_equal(W[3], np.array([0, 0, 0, 1], np.float32)), W
    # projection sparsity: rows (a,0,0,0), (0,c,0,0), (0,0,e,f), (0,0,-1,0)
    assert Pm[0, 1] == 0.0 and Pm[0, 2] == 0.0 and Pm[0, 3] == 0.0, Pm
    assert Pm[1, 0] == 0.0 and Pm[1, 2] == 0.0 and Pm[1, 3] == 0.0, Pm
    assert Pm[2, 0] == 0.0 and Pm[2, 1] == 0.0, Pm
    assert np.array_equal(Pm[3], np.array([0, 0, -1, 0], np.float32)), Pm
    return dict(
        a=float(Pm[0, 0]),
        c=float(Pm[1, 1]),
        e=float(Pm[2, 2]),
        f=float(Pm[2, 3]),
        tz=float(W[2, 3]),
    )


def _build_program(consts):
    a_, c_, e_, f_, tz = (
        consts["a"], consts["c"], consts["e"], consts["f"], consts["tz"],
    )
    nc = bass.Bass()
    pos_d = nc.dram_tensor("pos", [NC_ELEMS, 3], _dt, kind="ExternalInput")
    scr_d = nc.dram_tensor("scr", [NC_ELEMS, 3], _dt, kind="ExternalInput")
    cov_d = nc.dram_tensor("cov", [NC_ELEMS, 9], _dt, kind="ExternalInput")
    p2_d = nc.dram_tensor("pos2d", [NC_ELEMS, 3], _dt, kind="ExternalOutput")
    c2_d = nc.dram_tensor("cov2d", [NC_ELEMS, 4], _dt, kind="ExternalOutput")
    mk_d = nc.dram_tensor("mask", [NC_ELEMS, 1], _u8, kind="ExternalOutput")

    with tile.TileContext(nc) as tc:
        with tc.tile_pool(name="io", bufs=2) as io, \
             tc.tile_pool(name="tmp", bufs=1) as tp:
            for t in range(NT):
                sl = slice(t * P * F, (t + 1) * P * F)

                pos_t = io.tile([P, 3 * F], _dt, tag="pos", name="pos_t")
                nc.sync.dma_start(
                    pos_t[:], pos_d[sl, :].rearrange("(p f) c -> p (f c)", p=P)
                )
                scr_t = io.tile([P, 3 * F], _dt, tag="scr", name="scr_t")
                nc.sync.dma_start(
                    scr_t[:], scr_d[sl, :].rearrange("(p f) c -> p (f c)", p=P)
                )
                cov_t = io.tile([P, 9 * F], _dt, tag="cov", name="cov_t")
                nc.sync.dma_start(
                    cov_t[:], cov_d[sl, :].rearrange("(p f) c -> p (f c)", p=P)
                )

                pos3 = pos_t[:].rearrange("p (f c) -> p f c", c=3)
                scr3 = scr_t[:].rearrange("p (f c) -> p f c", c=3)
                cov9 = cov_t[:].rearrange("p (f c) -> p f c", c=9)
                x = pos3[:, :, 0]
                y = pos3[:, :, 1]
                z = pos3[:, :, 2]
                s0 = scr3[:, :, 0]
                s1 = scr3[:, :, 1]
                s2 = scr3[:, :, 2]
                C00 = cov9[:, :, 0]
                C01 = cov9[:, :, 1]
                C02 = cov9[:, :, 2]
                C11 = cov9[:, :, 4]
                C12 = cov9[:, :, 5]
                C22 = cov9[:, :, 8]

                def T(tag):
                    t_ = tp.tile([P, F], _dt, tag=tag, name=tag)
                    return t_[:]

                # ---- projection: clip -> ndc ----
                # zn = -(z + tz) = w-coordinate of clip position
                zn = T("zn")
                nc.scalar.activation(zn, z, _Act.Copy, bias=-tz, scale=-1.0)
                w = T("w")
                nc.vector.tensor_scalar_max(w, zn, 1e-6)
                rw = T("rw")
                nc.vector.reciprocal(rw, w)
                n0 = T("n0")
                nc.vector.scalar_tensor_tensor(n0, x, a_, rw, _Alu.mult, _Alu.mult)
                n1 = T("n1")
                nc.vector.scalar_tensor_tensor(n1, y, c_, rw, _Alu.mult, _Alu.mult)
                c2t = T("c2t")
                nc.vector.tensor_scalar(c2t, zn, -e_, f_, _Alu.mult, _Alu.add)
                n2 = T("n2")
                nc.vector.tensor_mul(n2, c2t, rw)

                # ---- mask ----
                ab0 = T("ab0")
                nc.scalar.activation(ab0, n0, _Act.Abs)
                ab1 = T("ab1")
                nc.scalar.activation(ab1, n1, _Act.Abs)
                mx = T("mx")
                nc.vector.tensor_max(mx, ab0, ab1)
                cA = T("cA")
                nc.vector.tensor_scalar(cA, mx, RELAX, None, _Alu.is_le)
                cB = T("cB")
                nc.vector.tensor_scalar(cB, n2, 0.2, None, _Alu.is_ge)
                cC = T("cC")
                nc.vector.tensor_scalar(cC, n2, RELAX, None, _Alu.is_le)
                mAB = T("mAB")
                nc.vector.tensor_mul(mAB, cA, cB)
                maskf = T("maskf")
                nc.vector.tensor_mul(maskf, mAB, cC)

                mk_t = io.tile([P, F], _u8, tag="mk", name="mk_t")
                nc.vector.tensor_copy(mk_t[:], maskf)

                # ---- pos2d ----
                p2_t = io.tile([P, 3 * F], _dt, tag="p2", name="p2_t")
                p23 = p2_t[:].rearrange("p (f c) -> p f c", c=3)
                t0 = T("t0")
                nc.vector.tensor_add(t0, n0, s0)
                p0a = T("p0a")
                nc.scalar.activation(
                    p0a, t0, _Act.Copy, bias=0.5 * WIDTH, scale=0.5 * WIDTH
                )
                nc.vector.tensor_mul(p23[:, :, 0], p0a, maskf)
                t1 = T("t1")
                nc.vector.tensor_add(t1, n1, s1)
                p1a = T("p1a")
                nc.scalar.activation(
                    p1a, t1, _Act.Copy, bias=0.5 * HEIGHT, scale=-0.5 * HEIGHT
                )
                nc.vector.tensor_mul(p23[:, :, 1], p1a, maskf)
                t2 = T("t2")
                nc.vector.tensor_add(t2, n2, s2)
                nc.vector.tensor_mul(p23[:, :, 2], t2, maskf)

                # ---- cov2d ----
                # iz2 = 1/z^2 via magic-constant seed + 2 Newton iterations
                z2 = T("z2")
                nc.scalar.activation(z2, z, _Act.Square)
                y0 = T("y0")
                nc.vector.tensor_scalar(
                    y0.bitcast(mybir.dt.int32), z2.bitcast(mybir.dt.int32),
                    -1, 0x7EF311C3, _Alu.mult, _Alu.add,
                )
                e1 = T("e1")
                nc.vector.scalar_tensor_tensor(e1, z2, -1.0, y0, _Alu.mult, _Alu.mult)
                y1 = T("y1")
                nc.vector.scalar_tensor_tensor(y1, e1, 2.0, y0, _Alu.add, _Alu.mult)
                e2 = T("e2")
                nc.vector.scalar_tensor_tensor(e2, z2, -1.0, y1, _Alu.mult, _Alu.mult)
                iz2 = T("iz2")
                nc.vector.scalar_tensor_tensor(iz2, e2, 2.0, y1, _Alu.add, _Alu.mult)
                m2 = T("m2")
                nc.vector.tensor_mul(m2, iz2, maskf)
                x2 = T("x2")
                nc.scalar.activation(x2, x, _Act.Square)
                y2 = T("y2")
                nc.scalar.activation(y2, y, _Act.Square)

                c2o = io.tile([P, 4 * F], _dt, tag="c2o", name="c2o")
                c24 = c2o[:].rearrange("p (f c) -> p f c", c=4)

                q1 = T("q1")
                nc.vector.scalar_tensor_tensor(q1, x, -2.0, C02, _Alu.mult, _Alu.mult)
                q2 = T("q2")
                nc.vector.tensor_mul(q2, x2, C22)
                q3 = T("q3")
                nc.vector.tensor_add(q3, q1, C00)
                q4 = T("q4")
                nc.vector.tensor_add(q4, q3, q2)
                nc.vector.tensor_mul(c24[:, :, 0], q4, m2)

                r1 = T("r1")
                nc.vector.scalar_tensor_tensor(r1, y, -2.0, C12, _Alu.mult, _Alu.mult)
                r2 = T("r2")
                nc.vector.tensor_mul(r2, y2, C22)
                r3 = T("r3")
                nc.vector.tensor_add(r3, r1, C11)
                r4 = T("r4")
                nc.vector.tensor_add(r4, r3, r2)
                nc.vector.tensor_mul(c24[:, :, 3], r4, m2)

                xy = T("xy")
                nc.vector.tensor_mul(xy, x, y)
                u1 = T("u1")
                nc.vector.scalar_tensor_tensor(u1, y, -1.0, C02, _Alu.mult, _Alu.mult)
                u2 = T("u2")
                nc.vector.scalar_tensor_tensor(u2, x, -1.0, C12, _Alu.mult, _Alu.mult)
                u3 = T("u3")
                nc.vector.tensor_mul(u3, xy, C22)
                v1 = T("v1")
                nc.vector.tensor_add(v1, u1, C01)
                v2 = T("v2")
                nc.vector.tensor_add(v2, v1, u2)
                v3 = T("v3")
                nc.vector.tensor_add(v3, v2, u3)
                nc.vector.tensor_mul(c24[:, :, 1], v3, m2)
                nc.scalar.activation(c24[:, :, 2], c24[:, :, 1], _Act.Copy)

                nc.sync.dma_start(
                    p2_d[sl, :].rearrange("(p f) c -> p (f c)", p=P), p2_t[:]
                )
                nc.sync.dma_start(
                    c2_d[sl, :].rearrange("(p f) c -> p (f c)", p=P), c2o[:]
                )
                nc.sync.dma_start(
                    mk_d[sl, :].rearrange("(p f) c -> p (f c)", p=P), mk_t[:]
                )
    return nc


def _pad_shard(arr, fill=0.0):
    """Pad [N, ...] to N_PAD along axis 0 and split into per-core chunks."""
    n = arr.shape[0]
    out = np.full((N_PAD,) + arr.shape[1:], fill, dtype=arr.dtype)
    out[:n] = arr
    return [np.ascontiguousarray(out[c * NC_ELEMS:(c + 1) * NC_ELEMS])
            for c in range(N_CORES)]


def run_camera(screen_coords, pos3d, cov3d, world2model, projection, trace=False):
    screen_coords = np.asarray(screen_coords, dtype=np.float32)
    pos3d = np.asarray(pos3d, dtype=np.float32)
    cov3d = np.asarray(cov3d, dtype=np.float32)
    n = pos3d.shape[0]

    consts = _camera_consts(world2model, projection)
    nc = _build_program(consts)
    _split_excess_waits(nc)

    pos_sh = _pad_shard(pos3d)
    scr_sh = _pad_shard(screen_coords)
    cov_sh = _pad_shard(cov3d.reshape(n, 9))
    # benign padding for z to keep 1/z well-defined on pad elements
    for c in range(N_CORES):
        lo = c * NC_ELEMS
        hi = lo + NC_ELEMS
        if hi > n:
            k = max(n - lo, 0)
            pos_sh[c][k:, 2] = 1.0

    in_maps = [
        {"pos": pos_sh[c], "scr": scr_sh[c], "cov": cov_sh[c]}
        for c in range(N_CORES)
    ]
    res = run_bass_kernel_spmd(nc, in_maps, list(range(N_CORES)), trace=trace)

    pos2d = np.concatenate([res.results[c]["pos2d"] for c in range(N_CORES)])[:n]
    cov2d = np.concatenate([res.results[c]["cov2d"] for c in range(N_CORES)])[:n]
    mask = np.concatenate([res.results[c]["mask"] for c in range(N_CORES)])[:n, 0]
    return (
        pos2d,
        cov2d.reshape(n, 2, 2),
        mask.astype(bool),
        res,
    )


def kernel(screen_coords, pos3d, cov3d, world2model, projection):
    pos2d, cov2d, mask, _ = run_camera(
        screen_coords, pos3d, cov3d, world2model, projection, trace=False
    )
    return pos2d, cov2d, mask
